# revision 24
# baseline (speedup 1.0000x reference)
"""GraphSAGE-style pooling aggregator kernel for Trainium2 (8 NeuronCores).

Computes, for full inputs:
    h      = relu(neighbor_features @ w_pool + bias_pool)   # (n*k, dim)
    pooled = max(h.reshape(n, k, dim), axis=1)              # (n, dim)
    out    = relu(concat([src, pooled], -1) @ w + bias)     # (n, out)

Sharding: data-parallel over nodes. Core c gets nodes [c*1250, (c+1)*1250)
and the matching 40000 neighbor rows; weights replicated. No collectives.

Key identity: max_k relu(z_k + b) == relu(b + max_k z_k) (b per-feature,
relu monotone). So the whole PSUM drain is a *pure raw max* of z; bias and
relu apply once per node at the end. This removes the bias_pool DMA from
the first-drain critical path and lets the ACT drain be a plain Copy.

Design (memory-regime):
- Host-side: neighbor features transposed and cast to fp8 e3m4 (HBM load
  20.5 MB -> 5.1 MB per core); w_pool bf16 (mixed bf16 x e3m4 matmul is
  bit-exact on the PE, measured). bias_pool rides as column 128 of the
  w_pool DMA (one transfer, no 4-byte-packet straggler).
- Phase 1: w_pool stationary on the PE (redundant LDWEIGHTS stripped after
  Tile legalization), z^T in PSUM fp32.
- Drain (the wall; ACT ~1.0 ns/elem, DVE-TR 1.19-1.28, DVE tree 0.72):
    nodes [0:96):   2x 1536-col PSUM tiles -> ACT Copy to SBUF bf16,
                    then a 5-level DVE tensor_max tree (2x_1P).
    nodes [96:128): 2x 512-col PSUM tiles -> DVE reduce_max direct (1x),
                    bf16 out into the same praw tile.
  Final per group: ONE DVE tensor_scalar (add bias_pool AP, max 0) over
  praw[128,128] bf16 -> pooled (phase-2 lhsT).
- Phase 2 for group g is emitted one group later: 3 accumulating matmuls
  (src@wtop + pooled@wbot + ones33@bias33), ACT relu into pair staging.
  Its PSUM tile shares the 2-slot [128,512] ring with the DVE-drain tiles.
- DMA: fp8 neighbor blocks ramp small->large on the sync HWDGE queue,
  prefetched ~4 groups ahead; group 0 uses graded A-tile widths
  (512/1024/1536); w_pool+bias ride the scalar HWDGE queue first; output
  stores are deferred until every block load is queued, 2-group batched.
"""

import os

import numpy as np

N, K, DIM, OUT = 10000, 32, 128, 128
N_CORES = 8
NODES_PC = N // N_CORES          # 1250 nodes per core
ROWS_PC = NODES_PC * K           # 40000 neighbor rows per core

GROUP = 128                      # nodes per group
GCOLS = GROUP * K                # 4096 neighbor cols per full group
ATILE = 1536                     # ACT-drained PSUM tile (3 banks)
DTILE = 512                      # DVE-drained PSUM tile (1 bank)
MM = 512                         # cols per matmul (1 PSUM bank of fp32)
LAG = 1                          # groups between pooled ready and phase 2

# groups of (node0, n_nodes): 9 x 128 + 1 x 98
GROUPS = []
_n0 = 0
while _n0 < NODES_PC:
    _g = min(GROUP, NODES_PC - _n0)
    GROUPS.append((_n0, _g))
    _n0 += _g

# DMA blocks as col ranges: group 0 split fine (its tiles are graded),
# then one block per group so no group waits on a multi-group transfer.
BLOCK_RANGES = [(0, 1024), (1024, 2560), (2560, 4096)] + [
    (c, min(c + 4096, 40000)) for c in range(4096, 40000, 4096)
]


def _block_of_col(c):
    for b, (c0, c1) in enumerate(BLOCK_RANGES):
        if c0 <= c < c1:
            return b
    raise ValueError(c)


# first group that touches each block (for prefetch pacing)
FIRST_GROUP_OF_BLOCK = {}
for _g, (_n0, _gn) in enumerate(GROUPS):
    for _b in range(_block_of_col(_n0 * K), _block_of_col(_n0 * K + _gn * K - 1) + 1):
        FIRST_GROUP_OF_BLOCK.setdefault(_b, _g)

STRIP_LDW = os.environ.get("AGG_STRIP_LDW", "1") == "1"
# which engine applies the phase-2 relu: "act" or "dve"
P2_RELU = os.environ.get("AGG_P2_RELU", "act")
# neighbor-feature dtype: "e3" (fp8 e3m4) or "bf16"
X_DTYPE = os.environ.get("AGG_X_DTYPE", "e3")


def _strip_redundant_ldweights(nc, mybir):
    """Remove LDWEIGHTS that reload the already-loaded stationary operand.

    Runs after Tile scheduling/legalization, so it sees the final per-engine
    instruction order. move_matmul_waits_to_ldweights first folds matmul
    waits onto the paired LDWEIGHTS; only signature-identical LDWEIGHTS that
    carry no waits/updates are dropped, so semaphore structure is preserved.
    """
    nc.move_matmul_waits_to_ldweights()
    stripped = 0
    for blk in nc.main_func.blocks:
        loaded = None
        keep = []
        for i in blk.instructions:
            if isinstance(i, mybir.InstLdweights):
                a = i.ins[0]
                sig = (
                    a.memref,
                    a.offset,
                    str(a.ap),
                    str(a.dtype),
                    i.is_transpose,
                    i.tile_size,
                    i.tile_position,
                    str(i.perf_mode),
                )
                si = i.sync_info
                clean = si is None or (not si.on_wait and not si.on_update)
                if clean and loaded == sig:
                    stripped += 1
                    continue
                loaded = sig
                keep.append(i)
                continue
            if isinstance(i, mybir.InstMatmult) and i.is_transpose:
                loaded = None
            keep.append(i)
        blk.instructions[:] = keep
    return stripped


def _build_nc():
    import concourse.bacc as bacc
    import concourse.mybir as mybir
    import concourse.tile as tile

    f32 = mybir.dt.float32
    bf16 = mybir.dt.bfloat16
    xdt = mybir.dt.float8e3 if X_DTYPE == "e3" else bf16
    AX = mybir.AxisListType
    AF = mybir.ActivationFunctionType
    ALU = mybir.AluOpType

    nc = bacc.Bacc(target_bir_lowering=False)

    srcT = nc.declare_dram_parameter("srcT", [DIM, NODES_PC], bf16, isOutput=False)
    nbrT = nc.declare_dram_parameter("nbrT", [DIM, ROWS_PC], xdt, isOutput=False)
    # w_pool with bias_pool (bf16) appended as column 128
    w_pool = nc.declare_dram_parameter("w_pool", [DIM, DIM + 1], bf16, isOutput=False)
    w = nc.declare_dram_parameter("w", [2 * DIM, OUT], bf16, isOutput=False)
    ones33 = nc.declare_dram_parameter("ones33", [32, GROUP], bf16, isOutput=False)
    bias33 = nc.declare_dram_parameter("bias33", [32, OUT], bf16, isOutput=False)
    out = nc.declare_dram_parameter("out", [NODES_PC, OUT], f32, isOutput=True)

    max_blk = max(c1 - c0 for c0, c1 in BLOCK_RANGES)

    with tile.TileContext(nc) as tc:
        with (
            tc.tile_pool(name="consts", bufs=1) as consts,
            tc.tile_pool(name="xt", bufs=12) as xt_pool,
            tc.tile_pool(name="hcp", bufs=3) as hcp_pool,
            tc.tile_pool(name="tree", bufs=3) as tree_pool,
            tc.tile_pool(name="praw", bufs=3) as praw_pool,
            tc.tile_pool(name="pooled", bufs=4) as pooled_pool,
            tc.tile_pool(name="outio", bufs=3) as outio,
            tc.tile_pool(name="ps_a", bufs=2, space="PSUM") as ps_a,
            tc.tile_pool(name="ps_m", bufs=2, space="PSUM") as ps_m,
        ):
            # --- w_pool(+bias col) first, then the first neighbor blocks,
            # then the phase-2-only constants (needed much later) ---
            # sync-queue head: w_pool gates the very first matmul. (The
            # scalar queue would stall it behind the hoisted ACT_TABLE_LOAD;
            # the gpsimd SWDGE adds 2-4us of Q7 descriptor latency.)
            wpool_sb = consts.tile([DIM, DIM + 1], bf16)
            nc.sync.dma_start(out=wpool_sb, in_=w_pool[:, :])
            zcol = consts.tile([DIM, 1], f32)
            nc.vector.memset(zcol, 0.0)
            # fp32 copy of the bias column (tensor_scalar wants fp32 scalars)
            bpool_f32 = consts.tile([DIM, 1], f32)
            nc.vector.tensor_copy(bpool_f32, wpool_sb[:, DIM : DIM + 1])

            xt_tiles = [None] * len(BLOCK_RANGES)

            def load_block(bi):
                if bi >= len(BLOCK_RANGES) or xt_tiles[bi] is not None:
                    return
                c0, c1 = BLOCK_RANGES[bi]
                xT = xt_pool.tile([DIM, max_blk], xdt, tag="xT", name="xT")
                nc.sync.dma_start(out=xT[:, : c1 - c0], in_=nbrT[:, c0:c1])
                xt_tiles[bi] = xT

            # all block descriptors upfront: 12 independent SBUF slots mean
            # none of these DMAs carries a wait, so the queue never stalls
            for _b in range(len(BLOCK_RANGES)):
                load_block(_b)

            wtop_sb = consts.tile([DIM, OUT], bf16)
            nc.sync.dma_start(out=wtop_sb, in_=w[0:DIM, :])
            wbot_sb = consts.tile([DIM, OUT], bf16)
            nc.sync.dma_start(out=wbot_sb, in_=w[DIM : 2 * DIM, :])
            ones_sb = consts.tile([32, GROUP], bf16)
            nc.sync.dma_start(out=ones_sb, in_=ones33[:, :])
            bias33_sb = consts.tile([32, OUT], bf16)
            nc.sync.dma_start(out=bias33_sb, in_=bias33[:, :])
            srcT_sb = consts.tile([DIM, NODES_PC], bf16)
            nc.sync.dma_start(out=srcT_sb, in_=srcT[:, :])

            pooled_tiles = [None] * len(GROUPS)
            pair_tiles = {}
            pending_stores = []

            def maybe_flush_stores(force=False):
                # stores ride the sync queue; only emit once every block
                # load is already queued so they can never delay a load
                if not force and any(t is None for t in xt_tiles):
                    return
                while pending_stores:
                    o_dst, o_src = pending_stores.pop(0)
                    nc.sync.dma_start(out=o_dst, in_=o_src)

            def emit_phase2(gs):
                """Phase 2 for a batch of consecutive groups: one PSUM bank,
                one relu drain, one batched store."""
                nq = len(gs)
                o_ps = ps_m.tile([DIM, MM], f32, tag="hd", name="o_ps")
                for q, g in enumerate(gs):
                    n0, gn = GROUPS[g]
                    pooled_t, poff = pooled_tiles[g]
                    sl = o_ps[:gn, q * OUT : q * OUT + OUT]
                    nc.tensor.matmul(
                        out=sl,
                        lhsT=srcT_sb[:, n0 : n0 + gn],
                        rhs=wtop_sb[:, :],
                        start=True,
                        stop=False,
                    )
                    nc.tensor.matmul(
                        out=sl,
                        lhsT=pooled_t[:, poff : poff + gn],
                        rhs=wbot_sb[:, :],
                        start=False,
                        stop=False,
                    )
                    nc.tensor.matmul(
                        out=sl,
                        lhsT=ones_sb[:, :gn],
                        rhs=bias33_sb[:, :],
                        start=False,
                        stop=True,
                    )
                o_st = outio.tile([GROUP, nq, OUT], f32, tag="opair", name="opair")
                full = all(GROUPS[g][1] == GROUP for g in gs)
                drains = (
                    [(0, nq * OUT, GROUP, o_st[:, :, :])]
                    if full
                    else [
                        (q * OUT, OUT, GROUPS[g][1], o_st[: GROUPS[g][1], q, :])
                        for q, g in enumerate(gs)
                    ]
                )
                for c0d, cw, gp, dst in drains:
                    if P2_RELU == "act":
                        # bias as an SBUF AP: a float bias would pull in the
                        # const-AP table and its per-engine preamble TENSOR_LOAD
                        nc.scalar.activation(
                            out=dst, in_=o_ps[:gp, c0d : c0d + cw], func=AF.Relu,
                            bias=zcol[:gp, :],
                        )
                    else:
                        nc.vector.tensor_scalar_max(
                            out=dst, in0=o_ps[:gp, c0d : c0d + cw], scalar1=0.0
                        )
                n0f, _ = GROUPS[gs[0]]
                n0l, gnl = GROUPS[gs[-1]]
                if gnl == GROUP:
                    pending_stores.append(
                        (
                            out[n0f : n0l + GROUP].rearrange(
                                "(q p) o -> p q o", p=GROUP
                            ),
                            o_st[:, :, :],
                        )
                    )
                else:
                    if nq > 1:
                        pending_stores.append(
                            (
                                out[n0f:n0l].rearrange("(q p) o -> p q o", p=GROUP),
                                o_st[:, : nq - 1, :],
                            )
                        )
                    pending_stores.append(
                        (out[n0l : n0l + gnl], o_st[:gnl, nq - 1, :])
                    )
                maybe_flush_stores()

            for gi, (n0, gn) in enumerate(GROUPS):
                c0 = n0 * K
                gc = gn * K

                def xsrc(col):
                    b = _block_of_col(col)
                    return xt_tiles[b], col - BLOCK_RANGES[b][0]

                # split: leading 512s -> DVE reduce_max (so the first TR can
                # start on block 0), trailing cols -> ACT Copy + tree. Group
                # 0 uses graded A-tile widths so ACT starts stepwise ASAP.
                # Groups pair up (0,1)..(6,7): one shared hcp/tree halves the
                # per-op DVE overhead; 8 and 9 run solo to keep the tail
                # short. Within a pair tile, half h = gi % 2.
                a_widths = [512, 1024, ATILE] if gi == 0 else [ATILE, ATILE]
                a_cols = sum(a_widths)
                a_nodes = a_cols // K
                d_nodes = gn - a_nodes
                d_cols = d_nodes * K
                paired = gi < 8
                h = gi % 2 if paired else 0
                nh = 2 if paired else 1  # groups per tree

                if h == 0:
                    hcp = hcp_pool.tile(
                        [DIM, nh * 2 * ATILE], bf16, tag="hcp", name="hcp"
                    )
                    praw = praw_pool.tile(
                        [DIM, nh * GROUP], bf16, tag="praw", name="praw"
                    )
                    pooled = pooled_pool.tile(
                        [DIM, nh * GROUP], bf16, tag="pooled", name="pooled"
                    )
                    cur_pair = (hcp, praw, pooled)
                else:
                    hcp, praw, pooled = cur_pair
                pooled_tiles[gi] = (pooled, h * GROUP)

                # --- DVE-drained tiles first: fused raw max over k ---
                t0 = 0
                while t0 < d_cols:
                    tw = min(DTILE, d_cols - t0)
                    hD = ps_m.tile([DIM, DTILE], f32, tag="hd", name="hD")
                    xS, xo = xsrc(c0 + t0)
                    nc.tensor.matmul(
                        out=hD[:, :tw],
                        lhsT=wpool_sb[:, 0:DIM],
                        rhs=xS[:, xo : xo + tw],
                        start=True,
                        stop=True,
                    )
                    rn0 = h * GROUP + t0 // K
                    nc.vector.reduce_max(
                        out=praw[:, rn0 : rn0 + tw // K],
                        in_=hD[:, :tw].rearrange("p (n k) -> p n k", k=K),
                        axis=AX.X,
                    )
                    t0 += tw

                # --- ACT-drained tiles: plain Copy (raw z) ---
                t0 = 0
                for tw_a in a_widths:
                    hT = ps_a.tile([DIM, ATILE], f32, tag="hA", name="hA")
                    for m0 in range(0, tw_a, MM):
                        xS, xo = xsrc(c0 + d_cols + t0 + m0)
                        nc.tensor.matmul(
                            out=hT[:, m0 : m0 + MM],
                            lhsT=wpool_sb[:, 0:DIM],
                            rhs=xS[:, xo : xo + MM],
                            start=True,
                            stop=True,
                        )
                    nc.scalar.activation(
                        out=hcp[:, h * 2 * ATILE + t0 : h * 2 * ATILE + t0 + tw_a],
                        in_=hT[:, :tw_a],
                        func=AF.Copy,
                    )
                    t0 += tw_a

                if h + 1 < nh:
                    continue  # tree + finish happen on the pair's 2nd group

                # --- bf16 max tree over the whole hcp (nh groups) ---
                m = nh * 96
                tA = tree_pool.tile([DIM, 192 * 16], bf16, tag="tA", name="tA")
                tB = tree_pool.tile([DIM, 192 * 8], bf16, tag="tB", name="tB")
                tC = tree_pool.tile([DIM, 192 * 4], bf16, tag="tC", name="tC")
                tD = tree_pool.tile([DIM, 192 * 2], bf16, tag="tD", name="tD")
                v = hcp[:, : nh * 2 * ATILE].rearrange("p (n k) -> p n k", k=K)
                a = tA[:, : m * 16].rearrange("p (n j) -> p n j", j=16)
                nc.vector.tensor_max(out=a, in0=v[:, :, 0:16], in1=v[:, :, 16:32])
                b = tB[:, : m * 8].rearrange("p (n j) -> p n j", j=8)
                nc.vector.tensor_max(out=b, in0=a[:, :, 0:8], in1=a[:, :, 8:16])
                cc = tC[:, : m * 4].rearrange("p (n j) -> p n j", j=4)
                nc.vector.tensor_max(out=cc, in0=b[:, :, 0:4], in1=b[:, :, 4:8])
                d = tD[:, : m * 2].rearrange("p (n j) -> p n j", j=2)
                nc.vector.tensor_max(out=d, in0=cc[:, :, 0:2], in1=cc[:, :, 2:4])
                # l5 lands at praw[:, q*GROUP + d_nodes : q*GROUP + 128]
                for q in range(nh):
                    p1 = praw[
                        :, q * GROUP + d_nodes : q * GROUP + d_nodes + 96
                    ].rearrange("p (n j) -> p n j", j=1)
                    dq = d[:, q * 96 : (q + 1) * 96, :]
                    nc.vector.tensor_max(
                        out=p1, in0=dq[:, :, 0:1], in1=dq[:, :, 1:2]
                    )

                # --- finish: pooled = relu(praw + bias_pool), one DVE op ---
                fin = (nh - 1) * GROUP + gn
                nc.vector.tensor_scalar(
                    out=pooled[:, :fin],
                    in0=praw[:, :fin],
                    scalar1=bpool_f32[:, :],
                    scalar2=0.0,
                    op0=ALU.add,
                    op1=ALU.max,
                )

                # phase 2 in quads once the quad's pooled tiles are ready
                if gi == 5:
                    emit_phase2([0, 1, 2, 3])
                elif gi == 8:
                    emit_phase2([4, 5, 6, 7])

            emit_phase2([8, 9])
            maybe_flush_stores(force=True)

    if STRIP_LDW:
        _strip_redundant_ldweights(nc, mybir)
    nc.compile()
    return nc


def _make_in_maps(inputs):
    import ml_dtypes

    bf = ml_dtypes.bfloat16
    xdt = ml_dtypes.float8_e3m4 if X_DTYPE == "e3" else bf
    src = np.asarray(inputs["src_features"], dtype=np.float32)
    nbr = np.asarray(inputs["neighbor_features"], dtype=np.float32)
    w_pool = np.asarray(inputs["w_pool"], dtype=np.float32)
    bias_pool = np.asarray(inputs["bias_pool"], dtype=np.float32)
    w = np.asarray(inputs["w"], dtype=np.float32).astype(bf)
    bias = np.asarray(inputs["bias"], dtype=np.float32)

    # w_pool (bf16) with bias_pool appended as column 128
    wp = np.concatenate([w_pool, bias_pool[:, None]], axis=1).astype(bf)

    ones33 = np.zeros((32, GROUP), dtype=np.float32)
    ones33[0, :] = 1.0
    bias33 = np.zeros((32, OUT), dtype=np.float32)
    bias33[0, :] = bias
    ones33 = ones33.astype(bf)
    bias33 = bias33.astype(bf)

    in_maps = []
    for c in range(N_CORES):
        in_maps.append(
            {
                "srcT": np.ascontiguousarray(
                    src[c * NODES_PC : (c + 1) * NODES_PC].T.astype(bf)
                ),
                "nbrT": np.ascontiguousarray(
                    nbr[c * ROWS_PC : (c + 1) * ROWS_PC].T.astype(xdt)
                ),
                "w_pool": wp,
                "ones33": ones33,
                "bias33": bias33,
                "w": w,
            }
        )
    return in_maps


_NC_CACHE = None


def kernel(**inputs: np.ndarray) -> np.ndarray:
    from concourse.bass_utils import run_bass_kernel_spmd

    global _NC_CACHE
    if _NC_CACHE is None:
        _NC_CACHE = _build_nc()
    nc = _NC_CACHE

    in_maps = _make_in_maps(inputs)
    res = run_bass_kernel_spmd(nc, in_maps, core_ids=list(range(N_CORES)))
    return np.concatenate([res.results[c]["out"] for c in range(N_CORES)], axis=0)
